# revision 17
# baseline (speedup 1.0000x reference)
"""Trainium2 Bass kernel for BANLayer (bilinear attention network layer).

Computation (per reference):
  v_ = relu(wn_linear(v));  q_ = relu(wn_linear(q))            # (B,NV,HK),(B,NQ,HK)
  att = einsum('hk,bvk,bqk->bhvq', h, v_, q_) + h_bias          # (B,8,NV,NQ)  [output]
  fusion = einsum('bvk,bhvq,bqk->bk', v_, att, q_)              # (B,HK)
  logits = avgpool_k3(fusion)*3 -> batchnorm(batch stats)       # (B,HD)       [output]

Strategy: data-parallel over batch (4 per core, 8 cores). bf16 matmuls with
fp32 PSUM accumulation. The head-summed attention A = sum_h att[b,h] is
computed as a 9th head (h_bar = sum_h h[h]). Fusion is computed K-major:
  S.T[k,q] = sum_v v_row[v,k] * A[v,q]   (PE)
  fusion[k] = sum_q q_T[k,q] * S.T[k,q]  (DVE fused mul+reduce)
AvgPool*k + partition-transpose handled by one sparse 0/1 fp32 matmul; the
BatchNorm batch stats (sum x, sum x^2) go through an 8-core AllReduce.
"""

import os
import sys

if "/opt/trn_rl_repo" not in sys.path:
    sys.path.insert(0, "/opt/trn_rl_repo")

import numpy as np
import ml_dtypes

import concourse.bass as bass
import concourse.mybir as mybir
import concourse.tile as tile
from concourse import bacc
from concourse.bass_utils import run_bass_kernel_spmd

# Problem dims
B, NV, NQ = 32, 256, 512
VD, QD, HD, K, HOUT = 512, 512, 512, 3, 8
HK = HD * K  # 1536
BN_EPS = 1e-5

N_CORES = 8
NB = B // N_CORES  # 4 local batches per core
KC = HK // 128     # 12 chunks of contraction/feature dim
KV = VD // 128     # 4 chunks of VD/QD
MV = NV // 128     # 2 chunks of NV
DC = HD // 128     # 4 chunks of HD
NH = HOUT + 1      # 8 heads + summed "9th head"

F32 = mybir.dt.float32
BF16 = mybir.dt.bfloat16
AF = mybir.ActivationFunctionType
ALU = mybir.AluOpType

bf16 = ml_dtypes.bfloat16


def _bc(ap, pos, count):
    """Insert a step-0 (broadcast) free dim at position pos (0 = first free dim)."""
    new = list(ap.ap)
    new.insert(1 + pos, [0, count])
    return bass.AP(ap.tensor, ap.offset, new)


def build_kernel():
    STAGE = int(os.environ.get("BK_STAGE", "5"))
    nc = bacc.Bacc()
    with tile.TileContext(nc) as tc:
        with (
            tc.tile_pool(name="dram", bufs=1, space="DRAM") as dram,
            tc.tile_pool(name="const", bufs=1) as const,
            tc.tile_pool(name="io", bufs=2) as io,
            tc.tile_pool(name="proj", bufs=2) as proj,
            tc.tile_pool(name="vhp", bufs=1) as vhp,
            tc.tile_pool(name="outp", bufs=4) as outp,
            tc.tile_pool(name="fus", bufs=1) as fusp,
            tc.tile_pool(name="scr", bufs=2) as scr,
            tc.tile_pool(name="bn", bufs=1) as bnp,
            tc.tile_pool(name="mm", bufs=6, space="PSUM") as mmp,
            tc.tile_pool(name="lg", bufs=1, space="PSUM") as lgp,
        ):
            # ---- DRAM I/O ----
            vT_d = dram.tile([NB, VD, NV], BF16, kind="ExternalInput", name="vT", uniquify=False)
            qT_d = dram.tile([NB, QD, NQ], BF16, kind="ExternalInput", name="qT", uniquify=False)
            wvT_d = dram.tile([VD, HK], BF16, kind="ExternalInput", name="wvT", uniquify=False)
            wqT_d = dram.tile([QD, HK], BF16, kind="ExternalInput", name="wqT", uniquify=False)
            vb_d = dram.tile([128, KC], F32, kind="ExternalInput", name="vb", uniquify=False)
            qb_d = dram.tile([128, KC], F32, kind="ExternalInput", name="qb", uniquify=False)
            vbrow_d = dram.tile([1, HK], BF16, kind="ExternalInput", name="vbrow", uniquify=False)
            h_d = dram.tile([128, KC, NH], F32, kind="ExternalInput", name="ht", uniquify=False)
            hb_d = dram.tile([128, NH], F32, kind="ExternalInput", name="hbt", uniquify=False)
            pp_d = dram.tile([KC, 128, HD], F32, kind="ExternalInput", name="ppool", uniquify=False)
            gam_d = dram.tile([128, DC], F32, kind="ExternalInput", name="gamma", uniquify=False)
            bet_d = dram.tile([128, DC], F32, kind="ExternalInput", name="beta", uniquify=False)

            att_d = dram.tile([NB, HOUT, NV, NQ], F32, kind="ExternalOutput", name="att_out", uniquify=False)
            lg_d = dram.tile([NB, HD], F32, kind="ExternalOutput", name="logits_out", uniquify=False)

            dbgv = dram.tile([128, KC, NV], BF16, name="dbgv", uniquify=False)
            dbgq = dram.tile([128, KC, NQ], BF16, name="dbgq", uniquify=False)
            dbgvr = dram.tile([128, MV, HK], BF16, name="dbgvr", uniquify=False)
            dbgA = dram.tile([128, MV, NQ], BF16, name="dbgA", uniquify=False)
            dbgf = dram.tile([128, KC, NB], F32, name="dbgf", uniquify=False)
            cc_in = dram.tile([128, 2 * DC], F32, name="cc_in", uniquify=False)
            cc_out = dram.tile([128, 2 * DC], F32, name="cc_out", uniquify=False, addr_space="Shared")

            # ---- load constants into SBUF ----
            # (chunked so the first projection matmuls start as soon as their
            # slice lands; pp is only needed at the very end)
            wv_sb = const.tile([128, KV, HK], BF16)
            wq_sb = const.tile([128, KV, HK], BF16)
            for kv in range(KV):
                nc.sync.dma_start(wv_sb[:, kv, :], wvT_d[kv * 128:(kv + 1) * 128, :].rearrange("p k -> p k"))
            for kv in range(KV):
                nc.sync.dma_start(wq_sb[:, kv, :], wqT_d[kv * 128:(kv + 1) * 128, :].rearrange("p k -> p k"))
            pp_sb = const.tile([128, KC, HD], F32)
            for kc3 in range(4):
                nc.sync.dma_start(pp_sb[:, kc3 * 3:(kc3 + 1) * 3, :],
                                  pp_d[kc3 * 3:(kc3 + 1) * 3].rearrange("c p d -> p c d"))
            vb_sb = const.tile([128, KC], F32)
            nc.sync.dma_start(vb_sb[:], vb_d[:])
            qb_sb = const.tile([128, KC], F32)
            nc.sync.dma_start(qb_sb[:], qb_d[:])
            vbrow_sb = const.tile([1, HK], BF16)
            nc.sync.dma_start(vbrow_sb[:], vbrow_d[:])
            h_sb = const.tile([128, KC, NH], F32)
            nc.sync.dma_start(h_sb[:], h_d[:])
            hb_sb = const.tile([128, NH], F32)
            nc.sync.dma_start(hb_sb[:], hb_d[:])
            gam_sb = const.tile([128, DC], F32)
            nc.sync.dma_start(gam_sb[:], gam_d[:])
            bet_sb = const.tile([128, DC], F32)
            nc.sync.dma_start(bet_sb[:], bet_d[:])
            ones1 = const.tile([1, 128], BF16)
            nc.vector.memset(ones1[:], 1.0)
            epst = const.tile([128, 1], F32)
            nc.vector.memset(epst[:], BN_EPS)

            # fusion columns: fus_sb[:, kc, b] = fusion[b, kc*128+p]
            fus_sb = fusp.tile([128, KC, NB], F32)
            lgps = lgp.tile([128, DC * NB], F32)

            for b in range(int(os.environ.get('BK_NB', NB))):
                vT_t = io.tile([128, KV, NV], BF16, tag="vin")
                nc.sync.dma_start(vT_t[:], vT_d[b].rearrange("(c p) n -> p c n", p=128))
                qT_t = io.tile([128, KV, NQ], BF16, tag="qin")
                nc.sync.dma_start(qT_t[:], qT_d[b].rearrange("(c p) n -> p c n", p=128))

                # K-major projections: v_sb[p, kc, n] = v_[b, n, kc*128+p]
                v_sb = proj.tile([128, KC, NV], BF16, tag="vsb")
                for m in range(KC):
                    ps = mmp.tile([128, NV], F32, tag="mm")
                    for kv in range(KV):
                        nc.tensor.matmul(
                            ps[:], wv_sb[:, kv, m * 128:(m + 1) * 128], vT_t[:, kv, :],
                            start=(kv == 0), stop=(kv == KV - 1))
                    nc.scalar.activation(v_sb[:, m, :], ps[:], AF.Relu, bias=vb_sb[:, m:m + 1])

                q_sb = proj.tile([128, KC, NQ], BF16, tag="qsb")
                for m in range(KC):
                    ps = mmp.tile([128, NQ], F32, tag="mm")
                    for kv in range(KV):
                        nc.tensor.matmul(
                            ps[:], wq_sb[:, kv, m * 128:(m + 1) * 128], qT_t[:, kv, :],
                            start=(kv == 0), stop=(kv == KV - 1))
                    nc.scalar.activation(q_sb[:, m, :], ps[:], AF.Relu, bias=qb_sb[:, m:m + 1])

                # row-major v_: vrow_sb[p, mv, k] = v_[b, mv*128+p, k]
                vrow_sb = proj.tile([128, MV, HK], BF16, tag="vrow")
                for m in range(MV):
                    for n3 in range(HK // 512):
                        ps = mmp.tile([128, 512], F32, tag="mm")
                        for kv in range(KV):
                            nc.tensor.matmul(
                                ps[:], vT_t[:, kv, m * 128:(m + 1) * 128],
                                wv_sb[:, kv, n3 * 512:(n3 + 1) * 512],
                                start=(kv == 0), stop=False)
                        # row-bias via K=1 matmul: out[m,n] += 1 * bias[n]
                        nc.tensor.matmul(
                            ps[:], ones1[:, :], vbrow_sb[:, n3 * 512:(n3 + 1) * 512],
                            start=False, stop=True)
                        nc.scalar.activation(vrow_sb[:, m, n3 * 512:(n3 + 1) * 512], ps[:], AF.Relu)

                if STAGE < 2:
                    nc.sync.dma_start(dbgv[:], v_sb[:])
                    nc.sync.dma_start(dbgq[:], q_sb[:])
                    nc.sync.dma_start(dbgvr[:], vrow_sb[:])
                    continue
                # vh[p, kc, j, n] = v_sb[p, kc, n] * h[j, kc*128+p]   (j=8 -> h_bar)
                vh_sb = vhp.tile([128, KC, NH, NV], BF16, tag="vh")
                for kc in range(KC):
                    for hh in range(NH):
                        nc.vector.tensor_scalar_mul(
                            vh_sb[:, kc, hh, :], v_sb[:, kc, :], h_sb[:, kc, hh:hh + 1])

                # attention maps (8 heads) + summed head A (j=8)
                A_sb = proj.tile([128, MV, NQ], BF16, tag="Asb")
                for hh in range(NH):
                    for m in range(MV):
                        ps = mmp.tile([128, NQ], F32, tag="mm")
                        for kc in range(KC):
                            nc.tensor.matmul(
                                ps[:], vh_sb[:, kc, hh, m * 128:(m + 1) * 128], q_sb[:, kc, :],
                                start=(kc == 0), stop=(kc == KC - 1))
                        if hh < HOUT:
                            ao = outp.tile([128, NQ], F32, tag="attout")
                            nc.scalar.activation(ao[:], ps[:], AF.Identity, bias=hb_sb[:, hh:hh + 1])
                            nc.sync.dma_start(att_d[b, hh, m * 128:(m + 1) * 128, :], ao[:])
                        else:
                            nc.scalar.activation(A_sb[:, m, :], ps[:], AF.Identity, bias=hb_sb[:, hh:hh + 1])

                if STAGE < 3:
                    nc.sync.dma_start(dbgvr[:], vrow_sb[:])
                    nc.sync.dma_start(dbgA[:], A_sb[:])
                    continue
                # S.T[p, mk, q] = sum_v vrow[v, mk*128+p] * A[v, q]; then
                # fusion[b, mk*128+p] = sum_q q_T[p, mk, q] * S.T[p, mk, q]
                for mk in range(KC):
                    ps = mmp.tile([128, NQ], F32, tag="mm")
                    for mv in range(MV):
                        nc.tensor.matmul(
                            ps[:], vrow_sb[:, mv, mk * 128:(mk + 1) * 128], A_sb[:, mv, :],
                            start=(mv == 0), stop=(mv == MV - 1))
                    prod = scr.tile([128, NQ], F32, tag="prod")
                    nc.vector.tensor_mul(prod[:], q_sb[:, mk, :], ps[:])
                    nc.vector.reduce_sum(fus_sb[:, mk, b:b + 1], prod[:], axis=mybir.AxisListType.X)
                if STAGE >= 4:
                    for dc in range(DC):
                        for kc in range(KC):
                            nc.tensor.matmul(
                                lgps[:, dc * NB + b:dc * NB + b + 1],
                                pp_sb[:, kc, dc * 128:(dc + 1) * 128], fus_sb[:, kc, b:b + 1],
                                start=(kc == 0), stop=(kc == KC - 1))

            # avgpool*K + transpose via sparse fp32 matmul: logits[p_d, b]
            if STAGE == 3:
                nc.sync.dma_start(dbgf[:], fus_sb[:])
            lg_all = bnp.tile([128, DC, NB], F32)
            for dc in range(DC if STAGE >= 4 else 0):
                nc.vector.tensor_copy(lg_all[:, dc, :], lgps[:, dc * NB:(dc + 1) * NB])

            # local BN stats: sum_b x, sum_b x^2
            S_sb = bnp.tile([128, 2 * DC], F32)
            for dc in range(DC if STAGE >= 4 else 0):
                nc.vector.reduce_sum(S_sb[:, dc:dc + 1], lg_all[:, dc, :], axis=mybir.AxisListType.X)
                sq = scr.tile([128, NB], F32, tag="sq")
                nc.vector.tensor_mul(sq[:], lg_all[:, dc, :], lg_all[:, dc, :])
                nc.vector.reduce_sum(S_sb[:, DC + dc:DC + dc + 1], sq[:], axis=mybir.AxisListType.X)

            if STAGE >= 5 and True:
                nc.sync.dma_start(cc_in[:], S_sb[:])
                nc.gpsimd.collective_compute(
                    "AllReduce", ALU.add,
                    replica_groups=[list(range(N_CORES))],
                    ins=[cc_in[:]], outs=[cc_out[:]])
                R_sb = bnp.tile([128, 2 * DC], F32)
                nc.sync.dma_start(R_sb[:], cc_out[:])
            else:
                R_sb = S_sb

            for dc in range(DC if STAGE >= 4 else 0):
                mu = bnp.tile([128, 1], F32, name=f"mu{dc}")
                nc.vector.tensor_scalar_mul(mu[:], R_sb[:, dc:dc + 1], 1.0 / B)
                e2 = bnp.tile([128, 1], F32, name=f"e2{dc}")
                nc.vector.tensor_scalar_mul(e2[:], R_sb[:, DC + dc:DC + dc + 1], 1.0 / B)
                mu2 = bnp.tile([128, 1], F32, name=f"mu2{dc}")
                nc.vector.tensor_mul(mu2[:], mu[:], mu[:])
                var = bnp.tile([128, 1], F32, name=f"var{dc}")
                nc.vector.tensor_sub(var[:], e2[:], mu2[:])
                srt = bnp.tile([128, 1], F32, name=f"srt{dc}")
                nc.scalar.activation(srt[:], var[:], AF.Sqrt, bias=epst[:])
                rstd = bnp.tile([128, 1], F32, name=f"rstd{dc}")
                nc.vector.reciprocal(rstd[:], srt[:])
                a = bnp.tile([128, 1], F32, name=f"a{dc}")
                nc.vector.tensor_mul(a[:], rstd[:], gam_sb[:, dc:dc + 1])
                mua = bnp.tile([128, 1], F32, name=f"mua{dc}")
                nc.vector.tensor_mul(mua[:], mu[:], a[:])
                sh = bnp.tile([128, 1], F32, name=f"sh{dc}")
                nc.vector.tensor_sub(sh[:], bet_sb[:, dc:dc + 1], mua[:])
                lo = outp.tile([128, NB], F32, tag="lgout", name=f"lo{dc}")
                nc.vector.tensor_scalar(lo[:], lg_all[:, dc, :], a[:], sh[:], ALU.mult, ALU.add)
                nc.sync.dma_start(
                    lg_d[:, dc * 128:(dc + 1) * 128].rearrange("b p -> p b"), lo[:])

    nc.compile()
    return nc


def prep_inputs(v, q, v_V, v_g, v_b, q_V, q_g, q_b, h_mat, h_bias, bn_gamma, bn_beta):
    """Host-side prep: weight-norm fold, transposes, layout, bf16 casts.
    Returns per-core input maps."""
    wv = (v_V * (np.float32(v_g) / np.linalg.norm(v_V))).astype(np.float32)
    wq = (q_V * (np.float32(q_g) / np.linalg.norm(q_V))).astype(np.float32)
    h = h_mat[0, :, 0, :].astype(np.float32)          # (8, HK)
    hb = h_bias[0, :, 0, 0].astype(np.float32)        # (8,)
    h9 = np.concatenate([h, h.sum(0, keepdims=True)], 0)       # (9, HK)
    hb9 = np.concatenate([hb, hb.sum(keepdims=True)], 0)       # (9,)

    wvT = np.ascontiguousarray(wv.T).astype(bf16)              # (VD, HK)
    wqT = np.ascontiguousarray(wq.T).astype(bf16)
    vb_t = np.ascontiguousarray(v_b.reshape(KC, 128).T).astype(np.float32)
    qb_t = np.ascontiguousarray(q_b.reshape(KC, 128).T).astype(np.float32)
    vbrow = v_b.reshape(1, HK).astype(bf16)
    # h_t[p, kc, j] = h9[j, kc*128+p]
    h_t = np.ascontiguousarray(h9.T.reshape(KC, 128, NH).transpose(1, 0, 2)).astype(np.float32)
    hb_t = np.broadcast_to(hb9, (128, NH)).copy().astype(np.float32)
    # pooling matrix: pp[kc, p, d] = 1 if (kc*128+p)//K == d
    idx = np.arange(HK) // K
    pp = np.zeros((HK, HD), np.float32)
    pp[np.arange(HK), idx] = 1.0
    pp_t = np.ascontiguousarray(pp.reshape(KC, 128, HD))
    gam_t = np.ascontiguousarray(bn_gamma.reshape(DC, 128).T).astype(np.float32)
    bet_t = np.ascontiguousarray(bn_beta.reshape(DC, 128).T).astype(np.float32)

    shared = {
        "wvT": wvT, "wqT": wqT, "vb": vb_t, "qb": qb_t, "vbrow": vbrow,
        "ht": h_t, "hbt": hb_t, "ppool": pp_t, "gamma": gam_t, "beta": bet_t,
    }
    in_maps = []
    for c in range(N_CORES):
        sl = slice(c * NB, (c + 1) * NB)
        vT = np.ascontiguousarray(v[sl].transpose(0, 2, 1)).astype(bf16)
        qT = np.ascontiguousarray(q[sl].transpose(0, 2, 1)).astype(bf16)
        in_maps.append({"vT": vT, "qT": qT, **shared})
    return in_maps


_NC_CACHE = None


def _get_nc():
    global _NC_CACHE
    if _NC_CACHE is None:
        _NC_CACHE = build_kernel()
    return _NC_CACHE


def kernel(v, q, v_V, v_g, v_b, q_V, q_g, q_b, h_mat, h_bias, bn_gamma, bn_beta,
           _trace=False, _trace_kwargs=None):
    nc = _get_nc()
    in_maps = prep_inputs(v, q, v_V, v_g, v_b, q_V, q_g, q_b, h_mat, h_bias,
                          bn_gamma, bn_beta)
    res = run_bass_kernel_spmd(nc, in_maps, list(range(N_CORES)), trace=_trace,
                               **(_trace_kwargs or {}))
    kernel.last_results = res
    logits = np.empty((B, HD), np.float32)
    att = np.empty((B, HOUT, NV, NQ), np.float32)
    for c in range(N_CORES):
        sl = slice(c * NB, (c + 1) * NB)
        logits[sl] = res.results[c]["logits_out"]
        att[sl] = res.results[c]["att_out"]
    return logits, att


# revision 18
# speedup vs baseline: 1.1286x; 1.1286x over previous
"""Trainium2 Bass kernel for BANLayer (bilinear attention network layer).

Computation (per reference):
  v_ = relu(wn_linear(v));  q_ = relu(wn_linear(q))            # (B,NV,HK),(B,NQ,HK)
  att = einsum('hk,bvk,bqk->bhvq', h, v_, q_) + h_bias          # (B,8,NV,NQ)  [output]
  fusion = einsum('bvk,bhvq,bqk->bk', v_, att, q_)              # (B,HK)
  logits = avgpool_k3(fusion)*3 -> batchnorm(batch stats)       # (B,HD)       [output]

Strategy: data-parallel over batch (4 per core, 8 cores). bf16 matmuls with
fp32 PSUM accumulation. The head-summed attention A = sum_h att[b,h] is
computed as a 9th head (h_bar = sum_h h[h]). Fusion is computed K-major:
  S.T[k,q] = sum_v v_row[v,k] * A[v,q]   (PE)
  fusion[k] = sum_q q_T[k,q] * S.T[k,q]  (DVE fused mul+reduce)
AvgPool*k + partition-transpose handled by one sparse 0/1 fp32 matmul; the
BatchNorm batch stats (sum x, sum x^2) go through an 8-core AllReduce.
"""

import os
import sys

if "/opt/trn_rl_repo" not in sys.path:
    sys.path.insert(0, "/opt/trn_rl_repo")

import numpy as np
import ml_dtypes

import concourse.bass as bass
import concourse.mybir as mybir
import concourse.tile as tile
from concourse import bacc
from concourse.bass_utils import run_bass_kernel_spmd

# Problem dims
B, NV, NQ = 32, 256, 512
VD, QD, HD, K, HOUT = 512, 512, 512, 3, 8
HK = HD * K  # 1536
BN_EPS = 1e-5

N_CORES = 8
NB = B // N_CORES  # 4 local batches per core
KC = HK // 128     # 12 chunks of contraction/feature dim
KV = VD // 128     # 4 chunks of VD/QD
MV = NV // 128     # 2 chunks of NV
DC = HD // 128     # 4 chunks of HD
NH = HOUT + 1      # 8 heads + summed "9th head"

F32 = mybir.dt.float32
BF16 = mybir.dt.bfloat16
AF = mybir.ActivationFunctionType
ALU = mybir.AluOpType

bf16 = ml_dtypes.bfloat16


def _bc(ap, pos, count):
    """Insert a step-0 (broadcast) free dim at position pos (0 = first free dim)."""
    new = list(ap.ap)
    new.insert(1 + pos, [0, count])
    return bass.AP(ap.tensor, ap.offset, new)


def build_kernel():
    STAGE = int(os.environ.get("BK_STAGE", "5"))
    nc = bacc.Bacc()
    with tile.TileContext(nc) as tc:
        with (
            tc.tile_pool(name="dram", bufs=1, space="DRAM") as dram,
            tc.tile_pool(name="const", bufs=1) as const,
            tc.tile_pool(name="io", bufs=2) as io,
            tc.tile_pool(name="proj", bufs=2) as proj,
            tc.tile_pool(name="vhp", bufs=1) as vhp,
            tc.tile_pool(name="outp", bufs=4) as outp,
            tc.tile_pool(name="fus", bufs=1) as fusp,
            tc.tile_pool(name="scr", bufs=2) as scr,
            tc.tile_pool(name="bn", bufs=1) as bnp,
            tc.tile_pool(name="mm", bufs=7, space="PSUM") as mmp,
            tc.tile_pool(name="lg", bufs=1, space="PSUM") as lgp,
        ):
            # ---- DRAM I/O ----
            vT_d = dram.tile([NB, VD, NV], BF16, kind="ExternalInput", name="vT", uniquify=False)
            qT_d = dram.tile([NB, QD, NQ], BF16, kind="ExternalInput", name="qT", uniquify=False)
            wvT_d = dram.tile([VD, HK], BF16, kind="ExternalInput", name="wvT", uniquify=False)
            wqT_d = dram.tile([QD, HK], BF16, kind="ExternalInput", name="wqT", uniquify=False)
            vb_d = dram.tile([128, KC], F32, kind="ExternalInput", name="vb", uniquify=False)
            qb_d = dram.tile([128, KC], F32, kind="ExternalInput", name="qb", uniquify=False)
            vbrow_d = dram.tile([1, HK], BF16, kind="ExternalInput", name="vbrow", uniquify=False)
            h_d = dram.tile([128, KC, NH], F32, kind="ExternalInput", name="ht", uniquify=False)
            hb_d = dram.tile([128, NH], F32, kind="ExternalInput", name="hbt", uniquify=False)
            pp_d = dram.tile([KC, 128, HD], F32, kind="ExternalInput", name="ppool", uniquify=False)
            gam_d = dram.tile([128, DC], F32, kind="ExternalInput", name="gamma", uniquify=False)
            bet_d = dram.tile([128, DC], F32, kind="ExternalInput", name="beta", uniquify=False)

            att_d = dram.tile([NB, HOUT, NV, NQ], F32, kind="ExternalOutput", name="att_out", uniquify=False)
            lg_d = dram.tile([NB, HD], F32, kind="ExternalOutput", name="logits_out", uniquify=False)

            dbgv = dram.tile([128, KC, NV], BF16, name="dbgv", uniquify=False)
            dbgq = dram.tile([128, KC, NQ], BF16, name="dbgq", uniquify=False)
            dbgvr = dram.tile([128, MV, HK], BF16, name="dbgvr", uniquify=False)
            dbgA = dram.tile([128, MV, NQ], BF16, name="dbgA", uniquify=False)
            dbgf = dram.tile([128, KC, NB], F32, name="dbgf", uniquify=False)
            cc_in = dram.tile([128, 2 * DC], F32, name="cc_in", uniquify=False)
            cc_out = dram.tile([128, 2 * DC], F32, name="cc_out", uniquify=False, addr_space="Shared")

            # ---- load constants into SBUF ----
            # (chunked so the first projection matmuls start as soon as their
            # slice lands; pp is only needed at the very end)
            wv_sb = const.tile([128, KV, HK], BF16)
            wq_sb = const.tile([128, KV, HK], BF16)
            for kv in range(KV):
                nc.sync.dma_start(wv_sb[:, kv, :], wvT_d[kv * 128:(kv + 1) * 128, :].rearrange("p k -> p k"))
            for kv in range(KV):
                nc.sync.dma_start(wq_sb[:, kv, :], wqT_d[kv * 128:(kv + 1) * 128, :].rearrange("p k -> p k"))
            pp_sb = const.tile([128, KC, HD], F32)
            for kc3 in range(4):
                nc.sync.dma_start(pp_sb[:, kc3 * 3:(kc3 + 1) * 3, :],
                                  pp_d[kc3 * 3:(kc3 + 1) * 3].rearrange("c p d -> p c d"))
            vb_sb = const.tile([128, KC], F32)
            nc.sync.dma_start(vb_sb[:], vb_d[:])
            qb_sb = const.tile([128, KC], F32)
            nc.sync.dma_start(qb_sb[:], qb_d[:])
            vbrow_sb = const.tile([1, HK], BF16)
            nc.sync.dma_start(vbrow_sb[:], vbrow_d[:])
            h_sb = const.tile([128, KC, NH], F32)
            nc.sync.dma_start(h_sb[:], h_d[:])
            hb_sb = const.tile([128, NH], F32)
            nc.sync.dma_start(hb_sb[:], hb_d[:])
            gam_sb = const.tile([128, DC], F32)
            nc.sync.dma_start(gam_sb[:], gam_d[:])
            bet_sb = const.tile([128, DC], F32)
            nc.sync.dma_start(bet_sb[:], bet_d[:])
            ones1 = const.tile([1, 128], BF16)
            nc.vector.memset(ones1[:], 1.0)
            epst = const.tile([128, 1], F32)
            nc.vector.memset(epst[:], BN_EPS)

            # fusion columns: fus_sb[:, kc, b] = fusion[b, kc*128+p]
            fus_sb = fusp.tile([128, KC, NB], F32)
            lgps = lgp.tile([128, DC * NB], F32)

            for b in range(int(os.environ.get('BK_NB', NB))):
                vT_t = io.tile([128, KV, NV], BF16, tag="vin")
                nc.sync.dma_start(vT_t[:], vT_d[b].rearrange("(c p) n -> p c n", p=128))
                qT_t = io.tile([128, KV, NQ], BF16, tag="qin")
                nc.sync.dma_start(qT_t[:], qT_d[b].rearrange("(c p) n -> p c n", p=128))

                # K-major projections: v_sb[p, kc, n] = v_[b, n, kc*128+p]
                v_sb = proj.tile([128, KC, NV], BF16, tag="vsb")
                for m in range(KC):
                    ps = mmp.tile([128, NV], F32, tag="mm")
                    for kv in range(KV):
                        nc.tensor.matmul(
                            ps[:], wv_sb[:, kv, m * 128:(m + 1) * 128], vT_t[:, kv, :],
                            start=(kv == 0), stop=(kv == KV - 1))
                    nc.scalar.activation(v_sb[:, m, :], ps[:], AF.Relu, bias=vb_sb[:, m:m + 1])

                q_sb = proj.tile([128, KC, NQ], BF16, tag="qsb")
                for m in range(KC):
                    ps = mmp.tile([128, NQ], F32, tag="mm")
                    for kv in range(KV):
                        nc.tensor.matmul(
                            ps[:], wq_sb[:, kv, m * 128:(m + 1) * 128], qT_t[:, kv, :],
                            start=(kv == 0), stop=(kv == KV - 1))
                    nc.scalar.activation(q_sb[:, m, :], ps[:], AF.Relu, bias=qb_sb[:, m:m + 1])

                if STAGE < 2:
                    nc.sync.dma_start(dbgv[:], v_sb[:])
                    nc.sync.dma_start(dbgq[:], q_sb[:])
                    nc.sync.dma_start(dbgvr[:], vrow_sb[:])
                    continue
                # vh[p, kc, j, n] = v_sb[p, kc, n] * h[j, kc*128+p]   (j=8 -> h_bar)
                vh_sb = vhp.tile([128, KC, NH, NV], BF16, tag="vh")
                for hh in range(NH):
                    for kc in range(KC):
                        nc.vector.tensor_scalar_mul(
                            vh_sb[:, kc, hh, :], v_sb[:, kc, :], h_sb[:, kc, hh:hh + 1])

                # attention maps (8 heads) + summed head A (j=8)
                A_sb = proj.tile([128, MV, NQ], BF16, tag="Asb")
                for hh in range(NH):
                    for m in range(MV):
                        ps = mmp.tile([128, NQ], F32, tag="mm")
                        for kc in range(KC):
                            nc.tensor.matmul(
                                ps[:], vh_sb[:, kc, hh, m * 128:(m + 1) * 128], q_sb[:, kc, :],
                                start=(kc == 0), stop=(kc == KC - 1))
                        if hh < HOUT:
                            ao = outp.tile([128, NQ], F32, tag="attout")
                            nc.scalar.activation(ao[:], ps[:], AF.Identity, bias=hb_sb[:, hh:hh + 1])
                            nc.sync.dma_start(att_d[b, hh, m * 128:(m + 1) * 128, :], ao[:])
                        else:
                            nc.scalar.activation(A_sb[:, m, :], ps[:], AF.Identity, bias=hb_sb[:, hh:hh + 1])

                if STAGE < 3:
                    nc.sync.dma_start(dbgvr[:], vrow_sb[:])
                    nc.sync.dma_start(dbgA[:], A_sb[:])
                    continue
                # row-major v_: vrow_sb[p, mv, k] = v_[b, mv*128+p, k]
                vrow_sb = proj.tile([128, MV, HK], BF16, tag="vrow")
                for m in range(MV):
                    for n3 in range(HK // 512):
                        ps = mmp.tile([128, 512], F32, tag="mm")
                        for kv in range(KV):
                            nc.tensor.matmul(
                                ps[:], vT_t[:, kv, m * 128:(m + 1) * 128],
                                wv_sb[:, kv, n3 * 512:(n3 + 1) * 512],
                                start=(kv == 0), stop=False)
                        # row-bias via K=1 matmul: out[m,n] += 1 * bias[n]
                        nc.tensor.matmul(
                            ps[:], ones1[:, :], vbrow_sb[:, n3 * 512:(n3 + 1) * 512],
                            start=False, stop=True)
                        nc.scalar.activation(vrow_sb[:, m, n3 * 512:(n3 + 1) * 512], ps[:], AF.Relu)

                # S.T[p, mk, q] = sum_v vrow[v, mk*128+p] * A[v, q]; then
                # fusion[b, mk*128+p] = sum_q q_T[p, mk, q] * S.T[p, mk, q]
                for mk in range(KC):
                    ps = mmp.tile([128, NQ], F32, tag="mm")
                    for mv in range(MV):
                        nc.tensor.matmul(
                            ps[:], vrow_sb[:, mv, mk * 128:(mk + 1) * 128], A_sb[:, mv, :],
                            start=(mv == 0), stop=(mv == MV - 1))
                    prod = scr.tile([128, NQ], F32, tag="prod")
                    nc.vector.tensor_mul(prod[:], q_sb[:, mk, :], ps[:])
                    nc.vector.reduce_sum(fus_sb[:, mk, b:b + 1], prod[:], axis=mybir.AxisListType.X)


            # avgpool*K + transpose via sparse fp32 matmul: logits[p_d, b]
            if STAGE == 3:
                nc.sync.dma_start(dbgf[:], fus_sb[:])
            lg_all = bnp.tile([128, DC, NB], F32)
            for dc in range(DC if STAGE >= 4 else 0):
                for kc in range(KC):
                    nc.tensor.matmul(
                        lgps[:, dc * NB:(dc + 1) * NB], pp_sb[:, kc, dc * 128:(dc + 1) * 128], fus_sb[:, kc, :],
                        start=(kc == 0), stop=(kc == KC - 1))
                nc.vector.tensor_copy(lg_all[:, dc, :], lgps[:, dc * NB:(dc + 1) * NB])

            # local BN stats: sum_b x, sum_b x^2
            S_sb = bnp.tile([128, 2 * DC], F32)
            for dc in range(DC if STAGE >= 4 else 0):
                nc.vector.reduce_sum(S_sb[:, dc:dc + 1], lg_all[:, dc, :], axis=mybir.AxisListType.X)
                sq = scr.tile([128, NB], F32, tag="sq")
                nc.vector.tensor_mul(sq[:], lg_all[:, dc, :], lg_all[:, dc, :])
                nc.vector.reduce_sum(S_sb[:, DC + dc:DC + dc + 1], sq[:], axis=mybir.AxisListType.X)

            if STAGE >= 5 and True:
                nc.sync.dma_start(cc_in[:], S_sb[:])
                nc.gpsimd.collective_compute(
                    "AllReduce", ALU.add,
                    replica_groups=[list(range(N_CORES))],
                    ins=[cc_in[:]], outs=[cc_out[:]])
                R_sb = bnp.tile([128, 2 * DC], F32)
                nc.sync.dma_start(R_sb[:], cc_out[:])
            else:
                R_sb = S_sb

            for dc in range(DC if STAGE >= 4 else 0):
                mu = bnp.tile([128, 1], F32, name=f"mu{dc}")
                nc.vector.tensor_scalar_mul(mu[:], R_sb[:, dc:dc + 1], 1.0 / B)
                e2 = bnp.tile([128, 1], F32, name=f"e2{dc}")
                nc.vector.tensor_scalar_mul(e2[:], R_sb[:, DC + dc:DC + dc + 1], 1.0 / B)
                mu2 = bnp.tile([128, 1], F32, name=f"mu2{dc}")
                nc.vector.tensor_mul(mu2[:], mu[:], mu[:])
                var = bnp.tile([128, 1], F32, name=f"var{dc}")
                nc.vector.tensor_sub(var[:], e2[:], mu2[:])
                srt = bnp.tile([128, 1], F32, name=f"srt{dc}")
                nc.scalar.activation(srt[:], var[:], AF.Sqrt, bias=epst[:])
                rstd = bnp.tile([128, 1], F32, name=f"rstd{dc}")
                nc.vector.reciprocal(rstd[:], srt[:])
                a = bnp.tile([128, 1], F32, name=f"a{dc}")
                nc.vector.tensor_mul(a[:], rstd[:], gam_sb[:, dc:dc + 1])
                mua = bnp.tile([128, 1], F32, name=f"mua{dc}")
                nc.vector.tensor_mul(mua[:], mu[:], a[:])
                sh = bnp.tile([128, 1], F32, name=f"sh{dc}")
                nc.vector.tensor_sub(sh[:], bet_sb[:, dc:dc + 1], mua[:])
                lo = outp.tile([128, NB], F32, tag="lgout", name=f"lo{dc}")
                nc.vector.tensor_scalar(lo[:], lg_all[:, dc, :], a[:], sh[:], ALU.mult, ALU.add)
                nc.sync.dma_start(
                    lg_d[:, dc * 128:(dc + 1) * 128].rearrange("b p -> p b"), lo[:])

    nc.compile()
    return nc


def prep_inputs(v, q, v_V, v_g, v_b, q_V, q_g, q_b, h_mat, h_bias, bn_gamma, bn_beta):
    """Host-side prep: weight-norm fold, transposes, layout, bf16 casts.
    Returns per-core input maps."""
    wv = (v_V * (np.float32(v_g) / np.linalg.norm(v_V))).astype(np.float32)
    wq = (q_V * (np.float32(q_g) / np.linalg.norm(q_V))).astype(np.float32)
    h = h_mat[0, :, 0, :].astype(np.float32)          # (8, HK)
    hb = h_bias[0, :, 0, 0].astype(np.float32)        # (8,)
    h9 = np.concatenate([h, h.sum(0, keepdims=True)], 0)       # (9, HK)
    hb9 = np.concatenate([hb, hb.sum(keepdims=True)], 0)       # (9,)

    wvT = np.ascontiguousarray(wv.T).astype(bf16)              # (VD, HK)
    wqT = np.ascontiguousarray(wq.T).astype(bf16)
    vb_t = np.ascontiguousarray(v_b.reshape(KC, 128).T).astype(np.float32)
    qb_t = np.ascontiguousarray(q_b.reshape(KC, 128).T).astype(np.float32)
    vbrow = v_b.reshape(1, HK).astype(bf16)
    # h_t[p, kc, j] = h9[j, kc*128+p]
    h_t = np.ascontiguousarray(h9.T.reshape(KC, 128, NH).transpose(1, 0, 2)).astype(np.float32)
    hb_t = np.broadcast_to(hb9, (128, NH)).copy().astype(np.float32)
    # pooling matrix: pp[kc, p, d] = 1 if (kc*128+p)//K == d
    idx = np.arange(HK) // K
    pp = np.zeros((HK, HD), np.float32)
    pp[np.arange(HK), idx] = 1.0
    pp_t = np.ascontiguousarray(pp.reshape(KC, 128, HD))
    gam_t = np.ascontiguousarray(bn_gamma.reshape(DC, 128).T).astype(np.float32)
    bet_t = np.ascontiguousarray(bn_beta.reshape(DC, 128).T).astype(np.float32)

    shared = {
        "wvT": wvT, "wqT": wqT, "vb": vb_t, "qb": qb_t, "vbrow": vbrow,
        "ht": h_t, "hbt": hb_t, "ppool": pp_t, "gamma": gam_t, "beta": bet_t,
    }
    in_maps = []
    for c in range(N_CORES):
        sl = slice(c * NB, (c + 1) * NB)
        vT = np.ascontiguousarray(v[sl].transpose(0, 2, 1)).astype(bf16)
        qT = np.ascontiguousarray(q[sl].transpose(0, 2, 1)).astype(bf16)
        in_maps.append({"vT": vT, "qT": qT, **shared})
    return in_maps


_NC_CACHE = None


def _get_nc():
    global _NC_CACHE
    if _NC_CACHE is None:
        _NC_CACHE = build_kernel()
    return _NC_CACHE


def kernel(v, q, v_V, v_g, v_b, q_V, q_g, q_b, h_mat, h_bias, bn_gamma, bn_beta,
           _trace=False, _trace_kwargs=None):
    nc = _get_nc()
    in_maps = prep_inputs(v, q, v_V, v_g, v_b, q_V, q_g, q_b, h_mat, h_bias,
                          bn_gamma, bn_beta)
    res = run_bass_kernel_spmd(nc, in_maps, list(range(N_CORES)), trace=_trace,
                               **(_trace_kwargs or {}))
    kernel.last_results = res
    logits = np.empty((B, HD), np.float32)
    att = np.empty((B, HOUT, NV, NQ), np.float32)
    for c in range(N_CORES):
        sl = slice(c * NB, (c + 1) * NB)
        logits[sl] = res.results[c]["logits_out"]
        att[sl] = res.results[c]["att_out"]
    return logits, att


# revision 19
# speedup vs baseline: 1.4445x; 1.2799x over previous
"""Trainium2 Bass kernel for BANLayer (bilinear attention network layer).

Computation (per reference):
  v_ = relu(wn_linear(v));  q_ = relu(wn_linear(q))            # (B,NV,HK),(B,NQ,HK)
  att = einsum('hk,bvk,bqk->bhvq', h, v_, q_) + h_bias          # (B,8,NV,NQ)  [output]
  fusion = einsum('bvk,bhvq,bqk->bk', v_, att, q_)              # (B,HK)
  logits = avgpool_k3(fusion)*3 -> batchnorm(batch stats)       # (B,HD)       [output]

Strategy: data-parallel over batch (4 per core, 8 cores). bf16 matmuls with
fp32 PSUM accumulation. The head-summed attention A = sum_h att[b,h] is
computed as a 9th head (h_bar = sum_h h[h]). Fusion is computed K-major:
  S.T[k,q] = sum_v v_row[v,k] * A[v,q]   (PE)
  fusion[k] = sum_q q_T[k,q] * S.T[k,q]  (DVE fused mul+reduce)
AvgPool*k + partition-transpose handled by one sparse 0/1 fp32 matmul; the
BatchNorm batch stats (sum x, sum x^2) go through an 8-core AllReduce.
"""

import os
import sys

if "/opt/trn_rl_repo" not in sys.path:
    sys.path.insert(0, "/opt/trn_rl_repo")

import numpy as np
import ml_dtypes

import concourse.bass as bass
import concourse.mybir as mybir
import concourse.tile as tile
from concourse import bacc
from concourse.bass_utils import run_bass_kernel_spmd

# Problem dims
B, NV, NQ = 32, 256, 512
VD, QD, HD, K, HOUT = 512, 512, 512, 3, 8
HK = HD * K  # 1536
BN_EPS = 1e-5

N_CORES = 8
NB = B // N_CORES  # 4 local batches per core
KC = HK // 128     # 12 chunks of contraction/feature dim
KV = VD // 128     # 4 chunks of VD/QD
MV = NV // 128     # 2 chunks of NV
DC = HD // 128     # 4 chunks of HD
NH = HOUT + 1      # 8 heads + summed "9th head"

F32 = mybir.dt.float32
BF16 = mybir.dt.bfloat16
AF = mybir.ActivationFunctionType
ALU = mybir.AluOpType

bf16 = ml_dtypes.bfloat16


def _bc(ap, pos, count):
    """Insert a step-0 (broadcast) free dim at position pos (0 = first free dim)."""
    new = list(ap.ap)
    new.insert(1 + pos, [0, count])
    return bass.AP(ap.tensor, ap.offset, new)


def build_kernel():
    STAGE = int(os.environ.get("BK_STAGE", "4"))
    nc = bacc.Bacc()
    with tile.TileContext(nc) as tc:
        with (
            tc.tile_pool(name="dram", bufs=1, space="DRAM") as dram,
            tc.tile_pool(name="const", bufs=1) as const,
            tc.tile_pool(name="io", bufs=2) as io,
            tc.tile_pool(name="proj", bufs=2) as proj,
            tc.tile_pool(name="vhp", bufs=1) as vhp,
            tc.tile_pool(name="outp", bufs=4) as outp,
            tc.tile_pool(name="fus", bufs=1) as fusp,
            tc.tile_pool(name="scr", bufs=2) as scr,
            tc.tile_pool(name="bn", bufs=1) as bnp,
            tc.tile_pool(name="mm", bufs=7, space="PSUM") as mmp,
            tc.tile_pool(name="lg", bufs=1, space="PSUM") as lgp,
        ):
            # ---- DRAM I/O ----
            vT_d = dram.tile([NB, VD, NV], BF16, kind="ExternalInput", name="vT", uniquify=False)
            qT_d = dram.tile([NB, QD, NQ], BF16, kind="ExternalInput", name="qT", uniquify=False)
            wvT_d = dram.tile([VD, HK], BF16, kind="ExternalInput", name="wvT", uniquify=False)
            wqT_d = dram.tile([QD, HK], BF16, kind="ExternalInput", name="wqT", uniquify=False)
            vb_d = dram.tile([128, KC], F32, kind="ExternalInput", name="vb", uniquify=False)
            qb_d = dram.tile([128, KC], F32, kind="ExternalInput", name="qb", uniquify=False)
            vbrow_d = dram.tile([1, HK], BF16, kind="ExternalInput", name="vbrow", uniquify=False)
            h_d = dram.tile([128, KC, NH], F32, kind="ExternalInput", name="ht", uniquify=False)
            hb_d = dram.tile([128, NH], F32, kind="ExternalInput", name="hbt", uniquify=False)
            pp_d = dram.tile([KC, 128, HD], F32, kind="ExternalInput", name="ppool", uniquify=False)
            gam_d = dram.tile([128, DC], F32, kind="ExternalInput", name="gamma", uniquify=False)
            bet_d = dram.tile([128, DC], F32, kind="ExternalInput", name="beta", uniquify=False)

            att_d = dram.tile([NB, HOUT, NV, NQ], F32, kind="ExternalOutput", name="att_out", uniquify=False)
            lg_d = dram.tile([NB, HD], F32, kind="ExternalOutput", name="logits_out", uniquify=False)

            dbgv = dram.tile([128, KC, NV], BF16, name="dbgv", uniquify=False)
            dbgq = dram.tile([128, KC, NQ], BF16, name="dbgq", uniquify=False)
            dbgvr = dram.tile([128, MV, HK], BF16, name="dbgvr", uniquify=False)
            dbgA = dram.tile([128, MV, NQ], BF16, name="dbgA", uniquify=False)
            dbgf = dram.tile([128, KC, NB], F32, name="dbgf", uniquify=False)
            cc_in = dram.tile([128, 2 * DC], F32, name="cc_in", uniquify=False)
            cc_out = dram.tile([128, 2 * DC], F32, name="cc_out", uniquify=False, addr_space="Shared")

            # ---- load constants into SBUF ----
            # (chunked so the first projection matmuls start as soon as their
            # slice lands; pp is only needed at the very end)
            wv_sb = const.tile([128, KV, HK], BF16)
            wq_sb = const.tile([128, KV, HK], BF16)
            for kv in range(KV):
                nc.sync.dma_start(wv_sb[:, kv, :], wvT_d[kv * 128:(kv + 1) * 128, :].rearrange("p k -> p k"))
            for kv in range(KV):
                nc.sync.dma_start(wq_sb[:, kv, :], wqT_d[kv * 128:(kv + 1) * 128, :].rearrange("p k -> p k"))
            pp_sb = const.tile([128, KC, HD], F32)
            for kc3 in range(4):
                nc.sync.dma_start(pp_sb[:, kc3 * 3:(kc3 + 1) * 3, :],
                                  pp_d[kc3 * 3:(kc3 + 1) * 3].rearrange("c p d -> p c d"))
            vb_sb = const.tile([128, KC], F32)
            nc.sync.dma_start(vb_sb[:], vb_d[:])
            qb_sb = const.tile([128, KC], F32)
            nc.sync.dma_start(qb_sb[:], qb_d[:])
            vbrow_sb = const.tile([1, HK], BF16)
            nc.sync.dma_start(vbrow_sb[:], vbrow_d[:])
            h_sb = const.tile([128, KC, NH], F32)
            nc.sync.dma_start(h_sb[:], h_d[:])
            hb_sb = const.tile([128, NH], F32)
            nc.sync.dma_start(hb_sb[:], hb_d[:])
            gam_sb = const.tile([128, DC], F32)
            nc.sync.dma_start(gam_sb[:], gam_d[:])
            bet_sb = const.tile([128, DC], F32)
            nc.sync.dma_start(bet_sb[:], bet_d[:])
            ones1 = const.tile([1, 128], BF16)
            nc.vector.memset(ones1[:], 1.0)
            epst = const.tile([128, 1], F32)
            nc.vector.memset(epst[:], BN_EPS)

            # fusion columns: fus_sb[:, kc, b] = fusion[b, kc*128+p]
            fus_sb = fusp.tile([128, KC, NB], F32)
            lgps = lgp.tile([128, DC * NB], F32)

            for b in range(int(os.environ.get('BK_NB', NB))):
                vT_t = io.tile([128, KV, NV], BF16, tag="vin")
                nc.sync.dma_start(vT_t[:], vT_d[b].rearrange("(c p) n -> p c n", p=128))
                qT_t = io.tile([128, KV, NQ], BF16, tag="qin")
                nc.sync.dma_start(qT_t[:], qT_d[b].rearrange("(c p) n -> p c n", p=128))

                # K-major projections: v_sb[p, kc, n] = v_[b, n, kc*128+p]
                v_sb = proj.tile([128, KC, NV], BF16, tag="vsb")
                for m in range(KC):
                    ps = mmp.tile([128, NV], F32, tag="mm")
                    for kv in range(KV):
                        nc.tensor.matmul(
                            ps[:], wv_sb[:, kv, m * 128:(m + 1) * 128], vT_t[:, kv, :],
                            start=(kv == 0), stop=(kv == KV - 1))
                    nc.scalar.activation(v_sb[:, m, :], ps[:], AF.Relu, bias=vb_sb[:, m:m + 1])

                q_sb = proj.tile([128, KC, NQ], BF16, tag="qsb")
                for m in range(KC):
                    ps = mmp.tile([128, NQ], F32, tag="mm")
                    for kv in range(KV):
                        nc.tensor.matmul(
                            ps[:], wq_sb[:, kv, m * 128:(m + 1) * 128], qT_t[:, kv, :],
                            start=(kv == 0), stop=(kv == KV - 1))
                    nc.scalar.activation(q_sb[:, m, :], ps[:], AF.Relu, bias=qb_sb[:, m:m + 1])

                if STAGE < 2:
                    nc.sync.dma_start(dbgv[:], v_sb[:])
                    nc.sync.dma_start(dbgq[:], q_sb[:])
                    nc.sync.dma_start(dbgvr[:], vrow_sb[:])
                    continue
                # vh[p, kc, j, n] = v_sb[p, kc, n] * h[j, kc*128+p]   (j=8 -> h_bar)
                vh_sb = vhp.tile([128, KC, NH, NV], BF16, tag="vh")
                for hh in range(NH):
                    for kc in range(KC):
                        nc.vector.tensor_scalar_mul(
                            vh_sb[:, kc, hh, :], v_sb[:, kc, :], h_sb[:, kc, hh:hh + 1])

                # attention maps (8 heads) + summed head A (j=8)
                A_sb = proj.tile([128, MV, NQ], BF16, tag="Asb")
                for hh in range(NH):
                    for m in range(MV):
                        ps = mmp.tile([128, NQ], F32, tag="mm")
                        for kc in range(KC):
                            nc.tensor.matmul(
                                ps[:], vh_sb[:, kc, hh, m * 128:(m + 1) * 128], q_sb[:, kc, :],
                                start=(kc == 0), stop=(kc == KC - 1))
                        if hh < HOUT:
                            ao = outp.tile([128, NQ], F32, tag="attout")
                            nc.scalar.activation(ao[:], ps[:], AF.Identity, bias=hb_sb[:, hh:hh + 1])
                            nc.sync.dma_start(att_d[b, hh, m * 128:(m + 1) * 128, :], ao[:])
                        else:
                            nc.scalar.activation(A_sb[:, m, :], ps[:], AF.Identity, bias=hb_sb[:, hh:hh + 1])

                if STAGE < 3:
                    nc.sync.dma_start(dbgvr[:], vrow_sb[:])
                    nc.sync.dma_start(dbgA[:], A_sb[:])
                    continue
                # row-major v_: vrow_sb[p, mv, k] = v_[b, mv*128+p, k]
                vrow_sb = proj.tile([128, MV, HK], BF16, tag="vrow")
                for m in range(MV):
                    for n3 in range(HK // 512):
                        ps = mmp.tile([128, 512], F32, tag="mm")
                        for kv in range(KV):
                            nc.tensor.matmul(
                                ps[:], vT_t[:, kv, m * 128:(m + 1) * 128],
                                wv_sb[:, kv, n3 * 512:(n3 + 1) * 512],
                                start=(kv == 0), stop=False)
                        # row-bias via K=1 matmul: out[m,n] += 1 * bias[n]
                        nc.tensor.matmul(
                            ps[:], ones1[:, :], vbrow_sb[:, n3 * 512:(n3 + 1) * 512],
                            start=False, stop=True)
                        nc.scalar.activation(vrow_sb[:, m, n3 * 512:(n3 + 1) * 512], ps[:], AF.Relu)

                # S.T[p, mk, q] = sum_v vrow[v, mk*128+p] * A[v, q]; then
                # fusion[b, mk*128+p] = sum_q q_T[p, mk, q] * S.T[p, mk, q]
                for mk in range(KC):
                    ps = mmp.tile([128, NQ], F32, tag="mm")
                    for mv in range(MV):
                        nc.tensor.matmul(
                            ps[:], vrow_sb[:, mv, mk * 128:(mk + 1) * 128], A_sb[:, mv, :],
                            start=(mv == 0), stop=(mv == MV - 1))
                    prod = scr.tile([128, NQ], F32, tag="prod")
                    nc.vector.tensor_mul(prod[:], q_sb[:, mk, :], ps[:])
                    nc.vector.reduce_sum(fus_sb[:, mk, b:b + 1], prod[:], axis=mybir.AxisListType.X)


            # avgpool*K + transpose via sparse fp32 matmul: logits[p_d, b]
            if STAGE == 3:
                nc.sync.dma_start(dbgf[:], fus_sb[:])
            lg_all = bnp.tile([128, DC, NB], F32)
            for dc in range(DC if STAGE >= 4 else 0):
                for kc in range(KC):
                    nc.tensor.matmul(
                        lgps[:, dc * NB:(dc + 1) * NB], pp_sb[:, kc, dc * 128:(dc + 1) * 128], fus_sb[:, kc, :],
                        start=(kc == 0), stop=(kc == KC - 1))
                nc.vector.tensor_copy(lg_all[:, dc, :], lgps[:, dc * NB:(dc + 1) * NB])

            # local BN stats + collective + on-device BN (STAGE 5 only; default
            # is raw logits out + host BN -- a collective in the NEFF throttles
            # ALL matmuls by ~22%, measured 216 -> 263 ns/MM)
            if STAGE >= 5:
                S_sb = bnp.tile([128, 2 * DC], F32)
                for dc in range(DC):
                    nc.vector.reduce_sum(S_sb[:, dc:dc + 1], lg_all[:, dc, :], axis=mybir.AxisListType.X)
                    sq = scr.tile([128, NB], F32, tag="sq")
                    nc.vector.tensor_mul(sq[:], lg_all[:, dc, :], lg_all[:, dc, :])
                    nc.vector.reduce_sum(S_sb[:, DC + dc:DC + dc + 1], sq[:], axis=mybir.AxisListType.X)
                nc.sync.dma_start(cc_in[:], S_sb[:])
                nc.gpsimd.collective_compute(
                    "AllReduce", ALU.add,
                    replica_groups=[list(range(N_CORES))],
                    ins=[cc_in[:]], outs=[cc_out[:]])
                R_sb = bnp.tile([128, 2 * DC], F32)
                nc.sync.dma_start(R_sb[:], cc_out[:])
                for dc in range(DC):
                    mu = bnp.tile([128, 1], F32, name=f"mu{dc}")
                    nc.vector.tensor_scalar_mul(mu[:], R_sb[:, dc:dc + 1], 1.0 / B)
                    e2 = bnp.tile([128, 1], F32, name=f"e2{dc}")
                    nc.vector.tensor_scalar_mul(e2[:], R_sb[:, DC + dc:DC + dc + 1], 1.0 / B)
                    mu2 = bnp.tile([128, 1], F32, name=f"mu2{dc}")
                    nc.vector.tensor_mul(mu2[:], mu[:], mu[:])
                    var = bnp.tile([128, 1], F32, name=f"var{dc}")
                    nc.vector.tensor_sub(var[:], e2[:], mu2[:])
                    srt = bnp.tile([128, 1], F32, name=f"srt{dc}")
                    nc.scalar.activation(srt[:], var[:], AF.Sqrt, bias=epst[:])
                    rstd = bnp.tile([128, 1], F32, name=f"rstd{dc}")
                    nc.vector.reciprocal(rstd[:], srt[:])
                    a = bnp.tile([128, 1], F32, name=f"a{dc}")
                    nc.vector.tensor_mul(a[:], rstd[:], gam_sb[:, dc:dc + 1])
                    mua = bnp.tile([128, 1], F32, name=f"mua{dc}")
                    nc.vector.tensor_mul(mua[:], mu[:], a[:])
                    sh = bnp.tile([128, 1], F32, name=f"sh{dc}")
                    nc.vector.tensor_sub(sh[:], bet_sb[:, dc:dc + 1], mua[:])
                    lo = outp.tile([128, NB], F32, tag="lgout", name=f"lo{dc}")
                    nc.vector.tensor_scalar(lo[:], lg_all[:, dc, :], a[:], sh[:], ALU.mult, ALU.add)
                    nc.sync.dma_start(
                        lg_d[:, dc * 128:(dc + 1) * 128].rearrange("b p -> p b"), lo[:])
            else:
                for dc in range(DC if STAGE >= 4 else 0):
                    nc.sync.dma_start(
                        lg_d[:, dc * 128:(dc + 1) * 128].rearrange("b p -> p b"), lg_all[:, dc, :])

    nc.compile()
    return nc


def prep_inputs(v, q, v_V, v_g, v_b, q_V, q_g, q_b, h_mat, h_bias, bn_gamma, bn_beta):
    """Host-side prep: weight-norm fold, transposes, layout, bf16 casts.
    Returns per-core input maps."""
    wv = (v_V * (np.float32(v_g) / np.linalg.norm(v_V))).astype(np.float32)
    wq = (q_V * (np.float32(q_g) / np.linalg.norm(q_V))).astype(np.float32)
    h = h_mat[0, :, 0, :].astype(np.float32)          # (8, HK)
    hb = h_bias[0, :, 0, 0].astype(np.float32)        # (8,)
    h9 = np.concatenate([h, h.sum(0, keepdims=True)], 0)       # (9, HK)
    hb9 = np.concatenate([hb, hb.sum(keepdims=True)], 0)       # (9,)

    wvT = np.ascontiguousarray(wv.T).astype(bf16)              # (VD, HK)
    wqT = np.ascontiguousarray(wq.T).astype(bf16)
    vb_t = np.ascontiguousarray(v_b.reshape(KC, 128).T).astype(np.float32)
    qb_t = np.ascontiguousarray(q_b.reshape(KC, 128).T).astype(np.float32)
    vbrow = v_b.reshape(1, HK).astype(bf16)
    # h_t[p, kc, j] = h9[j, kc*128+p]
    h_t = np.ascontiguousarray(h9.T.reshape(KC, 128, NH).transpose(1, 0, 2)).astype(np.float32)
    hb_t = np.broadcast_to(hb9, (128, NH)).copy().astype(np.float32)
    # pooling matrix: pp[kc, p, d] = 1 if (kc*128+p)//K == d
    idx = np.arange(HK) // K
    pp = np.zeros((HK, HD), np.float32)
    pp[np.arange(HK), idx] = 1.0
    pp_t = np.ascontiguousarray(pp.reshape(KC, 128, HD))
    gam_t = np.ascontiguousarray(bn_gamma.reshape(DC, 128).T).astype(np.float32)
    bet_t = np.ascontiguousarray(bn_beta.reshape(DC, 128).T).astype(np.float32)

    shared = {
        "wvT": wvT, "wqT": wqT, "vb": vb_t, "qb": qb_t, "vbrow": vbrow,
        "ht": h_t, "hbt": hb_t, "ppool": pp_t, "gamma": gam_t, "beta": bet_t,
    }
    in_maps = []
    for c in range(N_CORES):
        sl = slice(c * NB, (c + 1) * NB)
        vT = np.ascontiguousarray(v[sl].transpose(0, 2, 1)).astype(bf16)
        qT = np.ascontiguousarray(q[sl].transpose(0, 2, 1)).astype(bf16)
        in_maps.append({"vT": vT, "qT": qT, **shared})
    return in_maps


_NC_CACHE = None


def _get_nc():
    global _NC_CACHE
    if _NC_CACHE is None:
        _NC_CACHE = build_kernel()
    return _NC_CACHE


def kernel(v, q, v_V, v_g, v_b, q_V, q_g, q_b, h_mat, h_bias, bn_gamma, bn_beta,
           _trace=False, _trace_kwargs=None):
    nc = _get_nc()
    in_maps = prep_inputs(v, q, v_V, v_g, v_b, q_V, q_g, q_b, h_mat, h_bias,
                          bn_gamma, bn_beta)
    res = run_bass_kernel_spmd(nc, in_maps, list(range(N_CORES)), trace=_trace,
                               **(_trace_kwargs or {}))
    kernel.last_results = res
    logits = np.empty((B, HD), np.float32)
    att = np.empty((B, HOUT, NV, NQ), np.float32)
    for c in range(N_CORES):
        sl = slice(c * NB, (c + 1) * NB)
        logits[sl] = res.results[c]["logits_out"]
        att[sl] = res.results[c]["att_out"]
    if int(os.environ.get("BK_STAGE", "4")) < 5:
        # BatchNorm (train-mode batch stats) on host: 32x512 elements, trivial
        mu = logits.mean(0)
        var = np.mean((logits - mu) ** 2, axis=0)
        logits = ((logits - mu) / np.sqrt(var + BN_EPS) * bn_gamma + bn_beta).astype(np.float32)
    return logits, att


# revision 21
# speedup vs baseline: 1.6185x; 1.1205x over previous
"""Trainium2 Bass kernel for BANLayer (bilinear attention network layer).

Computation (per reference):
  v_ = relu(wn_linear(v));  q_ = relu(wn_linear(q))            # (B,NV,HK),(B,NQ,HK)
  att = einsum('hk,bvk,bqk->bhvq', h, v_, q_) + h_bias          # (B,8,NV,NQ)  [output]
  fusion = einsum('bvk,bhvq,bqk->bk', v_, att, q_)              # (B,HK)
  logits = avgpool_k3(fusion)*3 -> batchnorm(batch stats)       # (B,HD)       [output]

Strategy: data-parallel over batch (4 per core, 8 cores). bf16 matmuls with
fp32 PSUM accumulation. The head-summed attention A = sum_h att[b,h] is
computed as a 9th head (h_bar = sum_h h[h]). Fusion is computed K-major:
  S.T[k,q] = sum_v v_row[v,k] * A[v,q]   (PE; v_row = PE-transpose of v_T)
  fusion[k] = sum_q q_T[k,q] * S.T[k,q]  (DVE mul + free-axis reduce)
The kernel returns att_maps and the raw fusion vectors; the 3-wide avgpool
and BatchNorm (49K flops on (32,1536)) run on host. A device-side collective
for the BN batch stats was measured to throttle EVERY matmul in the NEFF
from 216 to 263 ns (collectives firmware active for the whole kernel), so
the all-reduce is deliberately avoided.
"""

import os
import sys

if "/opt/trn_rl_repo" not in sys.path:
    sys.path.insert(0, "/opt/trn_rl_repo")

import numpy as np
import ml_dtypes

import concourse.bass as bass
import concourse.mybir as mybir
import concourse.tile as tile
from concourse import bacc
from concourse.bass_utils import run_bass_kernel_spmd
from concourse.masks import make_identity

# Problem dims
B, NV, NQ = 32, 256, 512
VD, QD, HD, K, HOUT = 512, 512, 512, 3, 8
HK = HD * K  # 1536
BN_EPS = 1e-5

N_CORES = 8
NB = B // N_CORES  # 4 local batches per core
KC = HK // 128     # 12 chunks of contraction/feature dim
KV = VD // 128     # 4 chunks of VD/QD
MV = NV // 128     # 2 chunks of NV
NH = HOUT + 1      # 8 heads + summed "9th head"

F32 = mybir.dt.float32
BF16 = mybir.dt.bfloat16
AF = mybir.ActivationFunctionType
ALU = mybir.AluOpType

bf16 = ml_dtypes.bfloat16


def build_kernel():
    nc = bacc.Bacc()
    with tile.TileContext(nc) as tc:
        with (
            tc.tile_pool(name="dram", bufs=1, space="DRAM") as dram,
            tc.tile_pool(name="const", bufs=1) as const,
            tc.tile_pool(name="io", bufs=2) as io,
            tc.tile_pool(name="proj", bufs=2) as proj,
            tc.tile_pool(name="vhp", bufs=1) as vhp,
            tc.tile_pool(name="outp", bufs=4) as outp,
            tc.tile_pool(name="fus", bufs=1) as fusp,
            tc.tile_pool(name="scr", bufs=2) as scr,
            tc.tile_pool(name="mm", bufs=6, space="PSUM") as mmp,
            tc.tile_pool(name="tp", bufs=2, space="PSUM") as tpp,
        ):
            # ---- DRAM I/O ----
            vT_d = dram.tile([NB, VD, NV], BF16, kind="ExternalInput", name="vT", uniquify=False)
            qT_d = dram.tile([NB, QD, NQ], BF16, kind="ExternalInput", name="qT", uniquify=False)
            wvT_d = dram.tile([VD, HK], BF16, kind="ExternalInput", name="wvT", uniquify=False)
            wqT_d = dram.tile([QD, HK], BF16, kind="ExternalInput", name="wqT", uniquify=False)
            vb_d = dram.tile([128, KC], F32, kind="ExternalInput", name="vb", uniquify=False)
            qb_d = dram.tile([128, KC], F32, kind="ExternalInput", name="qb", uniquify=False)
            h_d = dram.tile([128, KC, NH], F32, kind="ExternalInput", name="ht", uniquify=False)
            hb_d = dram.tile([128, NH], F32, kind="ExternalInput", name="hbt", uniquify=False)

            att_d = dram.tile([NB, HOUT, NV, NQ], F32, kind="ExternalOutput", name="att_out", uniquify=False)
            fus_d = dram.tile([NB, HK], F32, kind="ExternalOutput", name="fus_out", uniquify=False)

            # ---- load constants into SBUF ----
            # First-needed data first, in small pieces, so the first projection
            # chain (wv[kv0] + vT[b0]) isn't bandwidth-starved by later loads.
            wv_sb = const.tile([128, KV, HK], BF16)
            wq_sb = const.tile([128, KV, HK], BF16)
            vT_t0 = io.tile([128, KV, NV], BF16, tag="vin")
            for s in range(4):
                nc.sync.dma_start(wv_sb[:, 0, s * 384:(s + 1) * 384],
                                  wvT_d[0:128, s * 384:(s + 1) * 384])
            for kv in range(KV):
                nc.sync.dma_start(vT_t0[:, kv, :],
                                  vT_d[0, kv * 128:(kv + 1) * 128, :])
            vb_sb = const.tile([128, KC], F32)
            nc.sync.dma_start(vb_sb[:], vb_d[:])
            qb_sb = const.tile([128, KC], F32)
            nc.sync.dma_start(qb_sb[:], qb_d[:])
            for kv in range(1, KV):
                nc.sync.dma_start(wv_sb[:, kv, :], wvT_d[kv * 128:(kv + 1) * 128, :])
            qT_t0 = io.tile([128, KV, NQ], BF16, tag="qin")
            for kv in range(KV):
                nc.sync.dma_start(qT_t0[:, kv, :],
                                  qT_d[0, kv * 128:(kv + 1) * 128, :])
            for kv in range(KV):
                nc.sync.dma_start(wq_sb[:, kv, :], wqT_d[kv * 128:(kv + 1) * 128, :])
            h_sb = const.tile([128, KC, NH], F32)
            nc.sync.dma_start(h_sb[:], h_d[:])
            hb_sb = const.tile([128, NH], F32)
            nc.sync.dma_start(hb_sb[:], hb_d[:])
            ident = const.tile([128, 128], BF16)
            make_identity(nc, ident)

            # fusion columns: fus_sb[p, b, kc] = fusion[b, kc*128+p]
            fus_sb = fusp.tile([128, NB, KC], F32)

            for b in range(NB):
                if b == 0:
                    vT_t, qT_t = vT_t0, qT_t0
                else:
                    vT_t = io.tile([128, KV, NV], BF16, tag="vin")
                    nc.sync.dma_start(vT_t[:], vT_d[b].rearrange("(c p) n -> p c n", p=128))
                    qT_t = io.tile([128, KV, NQ], BF16, tag="qin")
                    nc.sync.dma_start(qT_t[:], qT_d[b].rearrange("(c p) n -> p c n", p=128))

                # K-major projections: v_sb[p, kc, n] = v_[b, n, kc*128+p]
                v_sb = proj.tile([128, KC, NV], BF16, tag="vsb")
                for m in range(KC):
                    ps = mmp.tile([128, NV], F32, tag="mm")
                    for kv in range(KV):
                        nc.tensor.matmul(
                            ps[:], wv_sb[:, kv, m * 128:(m + 1) * 128], vT_t[:, kv, :],
                            start=(kv == 0), stop=(kv == KV - 1))
                    nc.scalar.activation(v_sb[:, m, :], ps[:], AF.Relu, bias=vb_sb[:, m:m + 1])

                q_sb = proj.tile([128, KC, NQ], BF16, tag="qsb")
                for m in range(KC):
                    ps = mmp.tile([128, NQ], F32, tag="mm")
                    for kv in range(KV):
                        nc.tensor.matmul(
                            ps[:], wq_sb[:, kv, m * 128:(m + 1) * 128], qT_t[:, kv, :],
                            start=(kv == 0), stop=(kv == KV - 1))
                    nc.scalar.activation(q_sb[:, m, :], ps[:], AF.Relu, bias=qb_sb[:, m:m + 1])

                # vh[p, kc, j, n] = v_sb[p, kc, n] * h[j, kc*128+p]   (j=8 -> h_bar)
                vh_sb = vhp.tile([128, KC, NH, NV], BF16, tag="vh")
                for hh in range(NH):
                    for kc in range(KC):
                        nc.vector.tensor_scalar_mul(
                            vh_sb[:, kc, hh, :], v_sb[:, kc, :], h_sb[:, kc, hh:hh + 1])

                # attention maps (8 heads) + summed head A (j=8)
                A_sb = proj.tile([128, MV, NQ], BF16, tag="Asb")
                for hh in range(NH):
                    for m in range(MV):
                        ps = mmp.tile([128, NQ], F32, tag="mm")
                        for kc in range(KC):
                            nc.tensor.matmul(
                                ps[:], vh_sb[:, kc, hh, m * 128:(m + 1) * 128], q_sb[:, kc, :],
                                start=(kc == 0), stop=(kc == KC - 1))
                        if hh < HOUT:
                            ao = outp.tile([128, NQ], F32, tag="attout")
                            nc.scalar.activation(ao[:], ps[:], AF.Identity, bias=hb_sb[:, hh:hh + 1])
                            nc.sync.dma_start(att_d[b, hh, m * 128:(m + 1) * 128, :], ao[:])
                        else:
                            nc.scalar.activation(A_sb[:, m, :], ps[:], AF.Identity, bias=hb_sb[:, hh:hh + 1])

                # v_row via PE transpose of v_T (identical post-relu bf16 values)
                vrow_sb = proj.tile([128, MV, HK], BF16, tag="vrow")
                for mv in range(MV):
                    for mk in range(KC):
                        tps = tpp.tile([128, 128], BF16, tag="tp")
                        nc.tensor.transpose(tps[:], v_sb[:, mk, mv * 128:(mv + 1) * 128], ident[:])
                        nc.scalar.activation(vrow_sb[:, mv, mk * 128:(mk + 1) * 128], tps[:],
                                             AF.Copy, bias=0.0)

                # S.T[p, mk, q] = sum_v vrow[v, mk*128+p] * A[v, q]; then
                # fusion[b, mk*128+p] = sum_q q_T[p, mk, q] * S.T[p, mk, q]
                for mk in range(KC):
                    ps = mmp.tile([128, NQ], F32, tag="mm")
                    for mv in range(MV):
                        nc.tensor.matmul(
                            ps[:], vrow_sb[:, mv, mk * 128:(mk + 1) * 128], A_sb[:, mv, :],
                            start=(mv == 0), stop=(mv == MV - 1))
                    prod = scr.tile([128, NQ], F32, tag="prod")
                    nc.vector.tensor_mul(prod[:], q_sb[:, mk, :], ps[:])
                    nc.vector.reduce_sum(fus_sb[:, b, mk:mk + 1], prod[:], axis=mybir.AxisListType.X)

            for b in range(NB):
                nc.sync.dma_start(fus_d[b].rearrange("(c p) -> p c", p=128), fus_sb[:, b, :])

    nc.compile()
    return nc


def prep_inputs(v, q, v_V, v_g, v_b, q_V, q_g, q_b, h_mat, h_bias, bn_gamma, bn_beta):
    """Host-side prep: weight-norm fold, transposes, layout, bf16 casts.
    Returns per-core input maps."""
    wv = (v_V * (np.float32(v_g) / np.linalg.norm(v_V))).astype(np.float32)
    wq = (q_V * (np.float32(q_g) / np.linalg.norm(q_V))).astype(np.float32)
    h = h_mat[0, :, 0, :].astype(np.float32)          # (8, HK)
    hb = h_bias[0, :, 0, 0].astype(np.float32)        # (8,)
    h9 = np.concatenate([h, h.sum(0, keepdims=True)], 0)       # (9, HK)
    hb9 = np.concatenate([hb, hb.sum(keepdims=True)], 0)       # (9,)

    wvT = np.ascontiguousarray(wv.T).astype(bf16)              # (VD, HK)
    wqT = np.ascontiguousarray(wq.T).astype(bf16)
    vb_t = np.ascontiguousarray(v_b.reshape(KC, 128).T).astype(np.float32)
    qb_t = np.ascontiguousarray(q_b.reshape(KC, 128).T).astype(np.float32)
    # h_t[p, kc, j] = h9[j, kc*128+p]
    h_t = np.ascontiguousarray(h9.T.reshape(KC, 128, NH).transpose(1, 0, 2)).astype(np.float32)
    hb_t = np.broadcast_to(hb9, (128, NH)).copy().astype(np.float32)

    shared = {"wvT": wvT, "wqT": wqT, "vb": vb_t, "qb": qb_t, "ht": h_t, "hbt": hb_t}
    in_maps = []
    for c in range(N_CORES):
        sl = slice(c * NB, (c + 1) * NB)
        vT = np.ascontiguousarray(v[sl].transpose(0, 2, 1)).astype(bf16)
        qT = np.ascontiguousarray(q[sl].transpose(0, 2, 1)).astype(bf16)
        in_maps.append({"vT": vT, "qT": qT, **shared})
    return in_maps


_NC_CACHE = None


def _get_nc():
    global _NC_CACHE
    if _NC_CACHE is None:
        _NC_CACHE = build_kernel()
    return _NC_CACHE


def kernel(v, q, v_V, v_g, v_b, q_V, q_g, q_b, h_mat, h_bias, bn_gamma, bn_beta,
           _trace=False, _trace_kwargs=None):
    nc = _get_nc()
    in_maps = prep_inputs(v, q, v_V, v_g, v_b, q_V, q_g, q_b, h_mat, h_bias,
                          bn_gamma, bn_beta)
    res = run_bass_kernel_spmd(nc, in_maps, list(range(N_CORES)), trace=_trace,
                               **(_trace_kwargs or {}))
    kernel.last_results = res
    fusion = np.empty((B, HK), np.float32)
    att = np.empty((B, HOUT, NV, NQ), np.float32)
    for c in range(N_CORES):
        sl = slice(c * NB, (c + 1) * NB)
        fusion[sl] = res.results[c]["fus_out"]
        att[sl] = res.results[c]["att_out"]
    # avgpool(k=3)*3 + BatchNorm (train-mode batch stats): 49K flops on host
    logits = fusion.reshape(B, HD, K).sum(-1)
    mu = logits.mean(0)
    var = np.mean((logits - mu) ** 2, axis=0)
    logits = ((logits - mu) / np.sqrt(var + BN_EPS) * bn_gamma + bn_beta).astype(np.float32)
    return logits, att


# revision 22
# speedup vs baseline: 1.7204x; 1.0629x over previous
"""Trainium2 Bass kernel for BANLayer (bilinear attention network layer).

Computation (per reference):
  v_ = relu(wn_linear(v));  q_ = relu(wn_linear(q))            # (B,NV,HK),(B,NQ,HK)
  att = einsum('hk,bvk,bqk->bhvq', h, v_, q_) + h_bias          # (B,8,NV,NQ)  [output]
  fusion = einsum('bvk,bhvq,bqk->bk', v_, att, q_)              # (B,HK)
  logits = avgpool_k3(fusion)*3 -> batchnorm(batch stats)       # (B,HD)       [output]

Strategy: data-parallel over batch (4 per core, 8 cores). bf16 matmuls with
fp32 PSUM accumulation. The head-summed attention A = sum_h att[b,h] is
computed as a 9th head (h_bar = sum_h h[h]). Fusion is computed K-major:
  S.T[k,q] = sum_v v_row[v,k] * A[v,q]   (PE; v_row = PE-transpose of v_T)
  fusion[k] = sum_q q_T[k,q] * S.T[k,q]  (DVE mul + free-axis reduce)
The kernel returns att_maps and the raw fusion vectors; the 3-wide avgpool
and BatchNorm (49K flops on (32,1536)) run on host. A device-side collective
for the BN batch stats was measured to throttle EVERY matmul in the NEFF
from 216 to 263 ns (collectives firmware active for the whole kernel), so
the all-reduce is deliberately avoided.
"""

import os
import sys

if "/opt/trn_rl_repo" not in sys.path:
    sys.path.insert(0, "/opt/trn_rl_repo")

import numpy as np
import ml_dtypes

import concourse.bass as bass
import concourse.mybir as mybir
import concourse.tile as tile
from concourse import bacc
from concourse.bass_utils import run_bass_kernel_spmd
from concourse.masks import make_identity

# Problem dims
B, NV, NQ = 32, 256, 512
VD, QD, HD, K, HOUT = 512, 512, 512, 3, 8
HK = HD * K  # 1536
BN_EPS = 1e-5

N_CORES = 8
NB = B // N_CORES  # 4 local batches per core
KC = HK // 128     # 12 chunks of contraction/feature dim
KV = VD // 128     # 4 chunks of VD/QD
MV = NV // 128     # 2 chunks of NV
NH = HOUT + 1      # 8 heads + summed "9th head"

F32 = mybir.dt.float32
BF16 = mybir.dt.bfloat16
AF = mybir.ActivationFunctionType
ALU = mybir.AluOpType

bf16 = ml_dtypes.bfloat16


def build_kernel():
    nc = bacc.Bacc()
    with tile.TileContext(nc) as tc:
        with (
            tc.tile_pool(name="dram", bufs=1, space="DRAM") as dram,
            tc.tile_pool(name="const", bufs=1) as const,
            tc.tile_pool(name="io", bufs=2) as io,
            tc.tile_pool(name="proj", bufs=2) as proj,
            tc.tile_pool(name="vhp", bufs=1) as vhp,
            tc.tile_pool(name="outp", bufs=4) as outp,
            tc.tile_pool(name="fus", bufs=1) as fusp,
            tc.tile_pool(name="scr", bufs=2) as scr,
            tc.tile_pool(name="mm", bufs=6, space="PSUM") as mmp,
            tc.tile_pool(name="tp", bufs=2, space="PSUM") as tpp,
        ):
            # ---- DRAM I/O ----
            vT_d = dram.tile([NB, VD, NV], BF16, kind="ExternalInput", name="vT", uniquify=False)
            qT_d = dram.tile([NB, QD, NQ], BF16, kind="ExternalInput", name="qT", uniquify=False)
            wvT_d = dram.tile([VD, HK], BF16, kind="ExternalInput", name="wvT", uniquify=False)
            wqT_d = dram.tile([QD, HK], BF16, kind="ExternalInput", name="wqT", uniquify=False)
            vb_d = dram.tile([128, KC], F32, kind="ExternalInput", name="vb", uniquify=False)
            qb_d = dram.tile([128, KC], F32, kind="ExternalInput", name="qb", uniquify=False)
            h_d = dram.tile([128, KC, NH], F32, kind="ExternalInput", name="ht", uniquify=False)
            hb_d = dram.tile([128, NH], F32, kind="ExternalInput", name="hbt", uniquify=False)

            att_d = dram.tile([NB, HOUT, NV, NQ], F32, kind="ExternalOutput", name="att_out", uniquify=False)
            fus_d = dram.tile([NB, HK], F32, kind="ExternalOutput", name="fus_out", uniquify=False)

            # ---- load constants into SBUF ----
            # First-needed data first, in small pieces, so the first projection
            # chain (wv[kv0] + vT[b0]) isn't bandwidth-starved by later loads.
            wv_sb = const.tile([128, KV, HK], BF16)
            wq_sb = const.tile([128, KV, HK], BF16)
            vT_t0 = io.tile([128, KV, NV], BF16, tag="vin")
            for s in range(4):
                nc.sync.dma_start(wv_sb[:, 0, s * 384:(s + 1) * 384],
                                  wvT_d[0:128, s * 384:(s + 1) * 384])
            for kv in range(KV):
                nc.sync.dma_start(vT_t0[:, kv, :],
                                  vT_d[0, kv * 128:(kv + 1) * 128, :])
            vb_sb = const.tile([128, KC], F32)
            nc.sync.dma_start(vb_sb[:], vb_d[:])
            qb_sb = const.tile([128, KC], F32)
            nc.sync.dma_start(qb_sb[:], qb_d[:])
            for kv in range(1, KV):
                nc.sync.dma_start(wv_sb[:, kv, :], wvT_d[kv * 128:(kv + 1) * 128, :])
            qT_t0 = io.tile([128, KV, NQ], BF16, tag="qin")
            for kv in range(KV):
                nc.sync.dma_start(qT_t0[:, kv, :],
                                  qT_d[0, kv * 128:(kv + 1) * 128, :])
            for kv in range(KV):
                nc.sync.dma_start(wq_sb[:, kv, :], wqT_d[kv * 128:(kv + 1) * 128, :])
            h_sb = const.tile([128, KC, NH], F32)
            nc.sync.dma_start(h_sb[:], h_d[:])
            hb_sb = const.tile([128, NH], F32)
            nc.sync.dma_start(hb_sb[:], hb_d[:])
            ident = const.tile([128, 128], BF16)
            make_identity(nc, ident)

            # fusion columns: fus_sb[p, b, kc] = fusion[b, kc*128+p]
            fus_sb = fusp.tile([128, NB, KC], F32)

            for b in range(NB):
                if b == 0:
                    vT_t, qT_t = vT_t0, qT_t0
                else:
                    vT_t = io.tile([128, KV, NV], BF16, tag="vin")
                    nc.sync.dma_start(vT_t[:], vT_d[b].rearrange("(c p) n -> p c n", p=128))
                    qT_t = io.tile([128, KV, NQ], BF16, tag="qin")
                    nc.sync.dma_start(qT_t[:], qT_d[b].rearrange("(c p) n -> p c n", p=128))

                # K-major projections: v_sb[p, kc, n] = v_[b, n, kc*128+p]
                v_sb = proj.tile([128, KC, NV], BF16, tag="vsb")
                for m in range(KC):
                    ps = mmp.tile([128, NV], F32, tag="mm")
                    for kv in range(KV):
                        nc.tensor.matmul(
                            ps[:], wv_sb[:, kv, m * 128:(m + 1) * 128], vT_t[:, kv, :],
                            start=(kv == 0), stop=(kv == KV - 1))
                    nc.scalar.activation(v_sb[:, m, :], ps[:], AF.Relu, bias=vb_sb[:, m:m + 1])

                q_sb = proj.tile([128, KC, NQ], BF16, tag="qsb")
                for m in range(KC):
                    ps = mmp.tile([128, NQ], F32, tag="mm")
                    for kv in range(KV):
                        nc.tensor.matmul(
                            ps[:], wq_sb[:, kv, m * 128:(m + 1) * 128], qT_t[:, kv, :],
                            start=(kv == 0), stop=(kv == KV - 1))
                    nc.scalar.activation(q_sb[:, m, :], ps[:], AF.Relu, bias=qb_sb[:, m:m + 1])

                # vh[p, kc, j, n] = v_sb[p, kc, n] * h[j, kc*128+p]
                vh_sb = vhp.tile([128, KC, HOUT, NV], BF16, tag="vh")
                for hh in range(HOUT):
                    for kc in range(KC):
                        nc.vector.tensor_scalar_mul(
                            vh_sb[:, kc, hh, :], v_sb[:, kc, :], h_sb[:, kc, hh:hh + 1])

                # attention maps; A = sum_h att[b,h] accumulated on DVE from the
                # fp32 head outputs (cheaper than a 9th head on PE, and exact)
                A_f32 = proj.tile([128, MV, NQ], F32, tag="Af32")
                A_sb = proj.tile([128, MV, NQ], BF16, tag="Asb")
                for hh in range(HOUT):
                    for m in range(MV):
                        ps = mmp.tile([128, NQ], F32, tag="mm")
                        for kc in range(KC):
                            nc.tensor.matmul(
                                ps[:], vh_sb[:, kc, hh, m * 128:(m + 1) * 128], q_sb[:, kc, :],
                                start=(kc == 0), stop=(kc == KC - 1))
                        ao = outp.tile([128, NQ], F32, tag="attout")
                        nc.scalar.activation(ao[:], ps[:], AF.Identity, bias=hb_sb[:, hh:hh + 1])
                        nc.sync.dma_start(att_d[b, hh, m * 128:(m + 1) * 128, :], ao[:])
                        if hh == 0:
                            nc.vector.tensor_copy(A_f32[:, m, :], ao[:])
                        elif hh < HOUT - 1:
                            nc.vector.tensor_add(A_f32[:, m, :], A_f32[:, m, :], ao[:])
                        else:
                            nc.vector.tensor_add(A_sb[:, m, :], A_f32[:, m, :], ao[:])

                # v_row via PE transpose of v_T (identical post-relu bf16 values)
                vrow_sb = proj.tile([128, MV, HK], BF16, tag="vrow")
                for mv in range(MV):
                    for mk in range(KC):
                        tps = tpp.tile([128, 128], BF16, tag="tp")
                        nc.tensor.transpose(tps[:], v_sb[:, mk, mv * 128:(mv + 1) * 128], ident[:])
                        nc.scalar.activation(vrow_sb[:, mv, mk * 128:(mk + 1) * 128], tps[:],
                                             AF.Copy, bias=0.0)

                # S.T[p, mk, q] = sum_v vrow[v, mk*128+p] * A[v, q]; then
                # fusion[b, mk*128+p] = sum_q q_T[p, mk, q] * S.T[p, mk, q]
                for mk in range(KC):
                    ps = mmp.tile([128, NQ], F32, tag="mm")
                    for mv in range(MV):
                        nc.tensor.matmul(
                            ps[:], vrow_sb[:, mv, mk * 128:(mk + 1) * 128], A_sb[:, mv, :],
                            start=(mv == 0), stop=(mv == MV - 1))
                    prod = scr.tile([128, NQ], F32, tag="prod")
                    nc.vector.tensor_mul(prod[:], q_sb[:, mk, :], ps[:])
                    nc.vector.reduce_sum(fus_sb[:, b, mk:mk + 1], prod[:], axis=mybir.AxisListType.X)

            for b in range(NB):
                nc.sync.dma_start(fus_d[b].rearrange("(c p) -> p c", p=128), fus_sb[:, b, :])

    nc.compile()
    return nc


def prep_inputs(v, q, v_V, v_g, v_b, q_V, q_g, q_b, h_mat, h_bias, bn_gamma, bn_beta):
    """Host-side prep: weight-norm fold, transposes, layout, bf16 casts.
    Returns per-core input maps."""
    wv = (v_V * (np.float32(v_g) / np.linalg.norm(v_V))).astype(np.float32)
    wq = (q_V * (np.float32(q_g) / np.linalg.norm(q_V))).astype(np.float32)
    h = h_mat[0, :, 0, :].astype(np.float32)          # (8, HK)
    hb = h_bias[0, :, 0, 0].astype(np.float32)        # (8,)
    h9 = np.concatenate([h, h.sum(0, keepdims=True)], 0)       # (9, HK)
    hb9 = np.concatenate([hb, hb.sum(keepdims=True)], 0)       # (9,)

    wvT = np.ascontiguousarray(wv.T).astype(bf16)              # (VD, HK)
    wqT = np.ascontiguousarray(wq.T).astype(bf16)
    vb_t = np.ascontiguousarray(v_b.reshape(KC, 128).T).astype(np.float32)
    qb_t = np.ascontiguousarray(q_b.reshape(KC, 128).T).astype(np.float32)
    # h_t[p, kc, j] = h9[j, kc*128+p]
    h_t = np.ascontiguousarray(h9.T.reshape(KC, 128, NH).transpose(1, 0, 2)).astype(np.float32)
    hb_t = np.broadcast_to(hb9, (128, NH)).copy().astype(np.float32)

    shared = {"wvT": wvT, "wqT": wqT, "vb": vb_t, "qb": qb_t, "ht": h_t, "hbt": hb_t}
    in_maps = []
    for c in range(N_CORES):
        sl = slice(c * NB, (c + 1) * NB)
        vT = np.ascontiguousarray(v[sl].transpose(0, 2, 1)).astype(bf16)
        qT = np.ascontiguousarray(q[sl].transpose(0, 2, 1)).astype(bf16)
        in_maps.append({"vT": vT, "qT": qT, **shared})
    return in_maps


_NC_CACHE = None


def _get_nc():
    global _NC_CACHE
    if _NC_CACHE is None:
        _NC_CACHE = build_kernel()
    return _NC_CACHE


def kernel(v, q, v_V, v_g, v_b, q_V, q_g, q_b, h_mat, h_bias, bn_gamma, bn_beta,
           _trace=False, _trace_kwargs=None):
    nc = _get_nc()
    in_maps = prep_inputs(v, q, v_V, v_g, v_b, q_V, q_g, q_b, h_mat, h_bias,
                          bn_gamma, bn_beta)
    res = run_bass_kernel_spmd(nc, in_maps, list(range(N_CORES)), trace=_trace,
                               **(_trace_kwargs or {}))
    kernel.last_results = res
    fusion = np.empty((B, HK), np.float32)
    att = np.empty((B, HOUT, NV, NQ), np.float32)
    for c in range(N_CORES):
        sl = slice(c * NB, (c + 1) * NB)
        fusion[sl] = res.results[c]["fus_out"]
        att[sl] = res.results[c]["att_out"]
    # avgpool(k=3)*3 + BatchNorm (train-mode batch stats): 49K flops on host
    logits = fusion.reshape(B, HD, K).sum(-1)
    mu = logits.mean(0)
    var = np.mean((logits - mu) ** 2, axis=0)
    logits = ((logits - mu) / np.sqrt(var + BN_EPS) * bn_gamma + bn_beta).astype(np.float32)
    return logits, att
